# revision 11
# baseline (speedup 1.0000x reference)
"""NT-Xent loss on 8 Trainium2 NeuronCores — fp8 DoubleRow, no-comm.

Full inputs in, full (scalar) output out. Row-parallel: core c owns
rows [1024c, 1024c+1024) and computes its 1024x8192 slab of the
similarity matrix. Inputs are row-rotated per core so the single SPMD
program sees its own rows at local positions 0..1023 (static diagonal
mask / positive-pair columns). No cross-core communication (PJRT
launch skew makes collectives unpredictable here); host sums the 8
per-core scalar partials.

Engine split per core:
  ACT  : exp(x/T) with fused row-sum accumulate (32 x 2048-wide),
         norm^2 psum->sbuf copies, final ln (lse). A dummy exp up
         front pulls the activation-table loads off the critical
         path.
  DVE  : feature squares (bf16 at 2x), rsqrt via the 0x5f3759df bit
         trick (replaces ACT Ln+Exp), fp8 quantized normalize mul,
         diagonal mask.
  GPS  : positive-pair elementwise products.
  PE   : fp8e4 DoubleRow sim matmuls (K=256 single pass at 2x rate),
         norm^2 ones-colsums, final cross-partition reduces.
"""
import numpy as np
import ml_dtypes

import concourse.bass as bass  # noqa: F401
import concourse.tile as tile
import concourse.bacc as bacc_mod
from concourse import bacc, mybir
from concourse.bass_utils import run_bass_kernel_spmd
from concourse.hw_specs import get_activation_tables as _real_tables

B, D = 4096, 256
N = 2 * B                # 8192 rows/cols of sim matrix
NCORES = 8
RPC = N // NCORES        # 1024 rows per core
TEMP = 0.07
SCALE = 1.0 / TEMP
KG = 2                   # contraction groups: D = 256 = 2 * 128
MT = RPC // 128          # 8 M-tiles per core
NG = 4                   # 2048-wide column groups per m-tile
NEG = -1.0e9

# rsqrt(x) ~= bitcast_i32(int(1597463007 - 0.5*int_view(x))) — the
# 0x5f3759df trick; |rel err| <= 3.4% on the norm, which perturbs the
# loss far less than the 2e-2 tolerance.
B_RSQ = 1597463007.0

AF = mybir.ActivationFunctionType
ALU = mybir.AluOpType
AX = mybir.AxisListType
PM = mybir.MatmulPerfMode
f32 = mybir.dt.float32
i32 = mybir.dt.int32
bf16 = mybir.dt.bfloat16
fp8 = mybir.dt.float8e4

# normalize pieces: (width, offset); first two small for fast start
PIECES = ((1024, 0), (1024, 1024), (2048, 2048), (2048, 4096),
          (2048, 6144))

_CACHE = {}


def _pinned_tables(arch):
    """Keep Exp/Ln only in natural_log_exp_and_others so the act-table
    insertion pass picks one set for the whole kernel (no reload)."""
    tables = _real_tables(arch)
    out = {}
    for name, funcs in tables.items():
        if name != "natural_log_exp_and_others":
            funcs = {f for f in funcs if f.name not in ("Exp", "Ln")}
        out[name] = funcs
    return out


def _build_nc():
    bacc_mod.get_activation_tables = _pinned_tables
    nc = bacc.Bacc("TRN2", target_bir_lowering=False, debug=False,
                   enable_asserts=False, num_devices=NCORES,
                   num_swdge_queues=2)

    zt_d = nc.dram_tensor("zt", [KG, 128, N], bf16, kind="ExternalInput")
    conesb_d = nc.dram_tensor("conesb", [128, KG, 128], bf16,
                              kind="ExternalInput")
    vonesf_d = nc.dram_tensor("vonesf", [128, 1], f32, kind="ExternalInput")
    negid_d = nc.dram_tensor("negid", [128, 128], f32, kind="ExternalInput")
    out_d = nc.dram_tensor("out", [1, 1], f32, kind="ExternalOutput")

    with tile.TileContext(nc) as tc:
        with (
            tc.tile_pool(name="singles", bufs=1) as singles,
            tc.tile_pool(name="sqp", bufs=2) as sqp,
            tc.tile_pool(name="invp", bufs=2) as invp,
            tc.tile_pool(name="expp", bufs=2) as expp,
            tc.tile_pool(name="ps", bufs=2, space="PSUM") as ps,
        ):
            # --- constants (SWDGE ring) ---
            conesb = singles.tile([128, KG, 128], bf16, tag="conesb")
            nc.gpsimd.dma_start(out=conesb, in_=conesb_d.ap())
            vonesf = singles.tile([128, 1], f32, tag="vonesf")
            nc.gpsimd.dma_start(out=vonesf, in_=vonesf_d.ap())
            negid = singles.tile([128, 128], f32, tag="negid")
            nc.gpsimd.dma_start(out=negid, in_=negid_d.ap())

            # --- features: piece-aligned DMAs over 3 rings ---
            zt = singles.tile([128, KG, N], bf16, tag="zt")
            zt_ap = zt_d.ap()
            for (w, off), eng in zip(
                    PIECES,
                    (nc.sync, nc.scalar, nc.gpsimd, nc.sync, nc.scalar)):
                eng.dma_start(
                    out=zt[:, :, off:off + w],
                    in_=zt_ap[:, :, off:off + w].rearrange("k p c -> p k c"))

            # dummy exp: trigger the act-table load before it matters
            warm = singles.tile([1, 2], f32, tag="warm")
            nc.vector.memset(warm, 1.0)
            nc.scalar.activation(warm[0:1, 0:1], warm[0:1, 1:2], AF.Exp)

            nf = singles.tile([128, KG, N], fp8, tag="nf")
            sums = singles.tile([128, MT * NG], f32, tag="sums")
            fin = singles.tile([128, 2], f32, tag="fin")

            def square(pi):
                w, off = PIECES[pi]
                sq = sqp.tile([128, KG, w], bf16, tag=f"sq{pi}",
                              name=f"sq{pi}")
                nc.vector.tensor_mul(sq, zt[:, :, off:off + w],
                                     zt[:, :, off:off + w])
                return sq

            def normalize(pi, sq):
                """ones-colsum of squares -> psum; ACT copy to SBUF;
                rsqrt bit trick on DVE; quantize nf to fp8."""
                w, off = PIECES[pi]
                nn = ps.tile([128, 2048], f32, tag="A", name=f"nn{pi}")
                for n in range(w // 512):
                    for kg in range(KG):
                        nc.tensor.matmul(
                            nn[:, 512 * n:512 * (n + 1)],
                            conesb[:, kg, :],
                            sq[:, kg, 512 * n:512 * (n + 1)],
                            start=(kg == 0), stop=(kg == KG - 1),
                            skip_group_check=True)
                nnsb = invp.tile([128, w], f32, tag=f"nnsb{w}",
                                 name=f"nnsb{pi}")
                nc.scalar.activation(nnsb, nn[:, 0:w], AF.Copy)
                inv = invp.tile([128, w], i32, tag=f"inv{w}",
                                name=f"inv{pi}")
                nc.vector.tensor_scalar(inv, nnsb[:].bitcast(i32),
                                        -0.5, B_RSQ,
                                        op0=ALU.mult, op1=ALU.add)
                invf = inv[:].bitcast(f32)
                for kg in range(KG):
                    nc.vector.tensor_mul(nf[:, kg, off:off + w],
                                         zt[:, kg, off:off + w], invf)

            def lhsT(m):
                return nf[:, :, 128 * m:128 * m + 128]

            def mg(m, g):
                """Emit group g (cols 2048g..2048g+2047) of m-tile m."""
                T = ps.tile([128, 2048], f32, tag="A", name=f"T{m}_{g}")
                for n in range(4):
                    nc.tensor.matmul(
                        T[:, 512 * n:512 * (n + 1)], lhsT(m),
                        nf[:, :, 2048 * g + 512 * n:2048 * g + 512 * (n + 1)],
                        start=True, stop=True, perf_mode=PM.DoubleRow,
                        skip_group_check=True)
                if g == 0:
                    sl = T[:, 128 * m:128 * m + 128]
                    nc.vector.tensor_add(sl, sl, negid)
                e = expp.tile([128, 2048], bf16, tag="e", name="e")
                idx = NG * m + g
                nc.scalar.activation(e, T, AF.Exp, scale=SCALE,
                                     accum_out=sums[:, idx:idx + 1])

            # startup: pieces 0-2 feed m=0 groups 0-1; pieces 3-4
            # normalize under the first exps. Emission order keeps the
            # psum ring alternating norm/sim generations.
            sq0 = square(0)
            sq1 = square(1)
            normalize(0, sq0)
            normalize(1, sq1)
            sq2 = square(2)
            normalize(2, sq2)
            mg(0, 0)
            mg(0, 1)
            sq3 = square(3)
            normalize(3, sq3)
            sq4 = square(4)
            normalize(4, sq4)
            mg(0, 2)
            mg(0, 3)

            # positive term on gpsimd: partner of row i is col i + 4096
            tmp_pos = sqp.tile([128, KG, RPC], bf16, tag="tpos")
            for kg in range(KG):
                nc.gpsimd.tensor_mul(tmp_pos[:, kg, :],
                                     nf[:, kg, 0:RPC],
                                     nf[:, kg, 4 * RPC:5 * RPC])

            for m in range(1, MT):
                for g in range(NG):
                    mg(m, g)
                if m == 3:
                    pos_ps = ps.tile([128, 2048], f32, tag="A",
                                     name="pos_ps")
                    for n in range(RPC // 512):
                        for kg in range(KG):
                            nc.tensor.matmul(
                                pos_ps[:, 512 * n:512 * (n + 1)],
                                conesb[:, kg, :],
                                tmp_pos[:, kg, 512 * n:512 * (n + 1)],
                                start=(kg == 0), stop=(kg == KG - 1),
                                skip_group_check=True)
                    nc.vector.tensor_reduce(fin[:, 1:2], pos_ps[:, 0:RPC],
                                            axis=AX.X, op=ALU.add)

            # --- finish: lse per row, reduce ---
            own = singles.tile([128, MT], f32, tag="own")
            nc.vector.tensor_reduce(
                own, sums.rearrange("p (m g) -> p m g", g=NG),
                axis=AX.X, op=ALU.add)
            lse8 = singles.tile([128, MT], f32, tag="lse8")
            nc.scalar.activation(lse8, own, AF.Ln)
            nc.vector.tensor_reduce(fin[:, 0:1], lse8, axis=AX.X,
                                    op=ALU.add)

            fin_ps = ps.tile([128, 2048], f32, tag="A", name="fin_ps")
            nc.tensor.matmul(fin_ps[0:1, 0:2], vonesf, fin,
                             start=True, stop=True, skip_group_check=True)
            # fin_ps[0,0] = sum_p lse_p ; fin_ps[0,1] = 128 * sum_i pos_i
            possc = singles.tile([1, 1], f32, tag="possc")
            nc.vector.tensor_scalar_mul(possc, fin_ps[0:1, 1:2],
                                        SCALE / 128.0)
            outv = singles.tile([1, 1], f32, tag="outv")
            nc.vector.tensor_sub(outv, fin_ps[0:1, 0:1], possc)
            nc.sync.dma_start(out=out_d.ap(), in_=outv)

    nc.compile()
    return nc


def _get_nc():
    if "nc" not in _CACHE:
        _CACHE["nc"] = _build_nc()
    return _CACHE["nc"]


def _in_maps(z_i, z_j):
    feats = np.concatenate([np.asarray(z_i, dtype=np.float32),
                            np.asarray(z_j, dtype=np.float32)], axis=0)
    conesb = np.ones((128, KG, 128), dtype=ml_dtypes.bfloat16)
    vonesf = np.ones((128, 1), dtype=np.float32)
    negid = (NEG * np.eye(128)).astype(np.float32)
    maps = []
    for c in range(NCORES):
        zr = np.roll(feats, -RPC * c, axis=0)           # [8192, 256]
        zq = zr.T.astype(ml_dtypes.bfloat16)            # [256, 8192]
        zt = np.ascontiguousarray(zq.reshape(KG, 128, N))
        maps.append({"zt": zt, "conesb": conesb, "vonesf": vonesf,
                     "negid": negid})
    return maps


def kernel(z_i, z_j, _trace=False, _trace_kwargs=None):
    nc = _get_nc()
    maps = _in_maps(z_i, z_j)
    res = run_bass_kernel_spmd(nc, maps, core_ids=list(range(NCORES)),
                               trace=_trace, **(_trace_kwargs or {}))
    total = sum(float(res.results[c]["out"][0, 0]) for c in range(NCORES))
    out = np.array(np.float32(total / N))
    if _trace:
        kernel._last_result = res
    return out
